# revision 17
# baseline (speedup 1.0000x reference)
"""DeepseekVL2 MoE gate (sigmoid + grouped top-k routing) on 8 trn2 cores.

Contract: kernel(**inputs) takes the FULL unsharded inputs
  hidden_states [4, 4096, 7168] f32, weight [256, 7168] f32,
  e_score_correction_bias [256] f32
and returns (topk_idx [16384, 8] int32, topk_weight [16384, 8] f32),
matching reference jax semantics.

Strategy (v4 — w-stationary, tapered chunks):
  - Data parallel: 16384 tokens -> 2048 per core x 8 cores, processed in
    chunks of [512, 512, 512, 256, 256] tokens. The tapered tail halves
    the after-last-matmul DVE routing drain (the 4 serial per-128-token
    routing chains of a 512 chunk were a ~23us tail; a 256 chunk leaves
    only 2, and the previous chunk's chains hide under its matmuls).
  - logits*1024 = xh16@wh16  (fp16)  +  2^-12 * (xl8@wh8 + xh8@wl8) (fp8)
    with xh16 = fp16(x), xl8 = e4m3((x-xh16)*4096), wh16 = fp16(w*1024),
    wh8 = e4m3(wh16), wl8 = e4m3((w*1024-wh16)*4096), xh8 = e4m3(xh16)
    converted on-chip by the ACT engine.
  - W-STATIONARY orientation: out = [128 experts, tc tokens]. 512-token
    moving streams (213ns fp16 / 107ns fp8-DoubleRow) hide every
    stationary load (137ns fp16 / 95ns fp8 pair); [token, expert]
    orientation is LDWEIGHTS-port-bound. fp8 DoubleRow streams at 1.0
    cycles/out-column on TRN2 (not the cost model's 0.5) — its win is
    2x contraction per instruction, making the correction 1.0 passes.
  - fp8 correction packs both residual terms of one k-tile in ONE
    DoubleRow matmul: lhsT = [wh8(k) | wl8(k)], rhs = [xl8(k) ; xh8(k)].
  - Unified k-pass per chunk (per k: main eh0, main eh1, DR eh0, DR eh1)
    so each x k-slice is consumed once and its ring buffer frees early.
  - logits transposed back to [token, expert] with PE transpose (fp32,
    128x128 blocks via identity), then sigmoid + routing per 128-token
    block on DVE (max8 / max_index / match_replace, jax tie semantics).
  - PSUM zero regions: matmul start=True zeroes the whole 2KB bank ->
    main / corr / transpose groups live in distinct full banks; packed
    transpose blocks use start=True only on the bank's first write.
"""

import os
import numpy as np
import ml_dtypes

import concourse.bacc as bacc
import concourse.bass as bass
import concourse.mybir as mybir
from concourse.bass_utils import run_bass_kernel_spmd
from concourse.tile import TileContext

F16 = mybir.dt.float16
F32 = mybir.dt.float32
F8 = mybir.dt.float8e4
U32 = mybir.dt.uint32
I32 = mybir.dt.int32
E4NP = ml_dtypes.float8_e4m3

N_CORES = 8
T_FULL = 16384
T_CORE = T_FULL // N_CORES          # 2048
H = 7168
E = 256
KT = H // 128                        # 56 contraction tiles
CHUNKS = [512, 512, 512, 256, 256]   # tokens per chunk (sum = 2048)
OFFS = [sum(CHUNKS[:i]) for i in range(len(CHUNKS))]
N_CHUNK = len(CHUNKS)
KS = 14                              # k-tiles per DMA slice
N_SLICE = KT // KS                   # 4 slices per chunk
N_GROUP = 8
GROUP_SIZE = E // N_GROUP            # 32
TOPK_GROUP = 4
TOP_K = 8
ROUTED_SCALING = 2.5
W_SCALE = 1024.0
S12 = 4096.0
NEG_BIG = -1.0e30


def _build_nc():
    nc = bacc.Bacc(
        "TRN2",
        target_bir_lowering=False,
        debug=False,
        num_devices=N_CORES,
    )

    # per-chunk x tensors keep every DMA destination one contiguous run
    # per partition (layout [p, k, t-within-chunk])
    xh_d = [
        nc.dram_tensor(f"xh{c}", [128, KT, CHUNKS[c]], F16, kind="ExternalInput").ap()
        for c in range(N_CHUNK)
    ]
    xl8_d = [
        nc.dram_tensor(f"xl8_{c}", [128, KT, CHUNKS[c]], F8, kind="ExternalInput").ap()
        for c in range(N_CHUNK)
    ]
    wh_d = nc.dram_tensor("wh16", [128, KT, E], F16, kind="ExternalInput").ap()
    w8_d = nc.dram_tensor("w8", [128, KT, 2, E], F8, kind="ExternalInput").ap()
    bias_d = nc.dram_tensor("biasb", [128, E], F32, kind="ExternalInput").ap()
    ident_d = nc.dram_tensor("ident", [128, 128], F32, kind="ExternalInput").ap()
    idx_d = nc.dram_tensor("out_idx", [T_CORE, TOP_K], I32, kind="ExternalOutput").ap()
    w_d = nc.dram_tensor("out_w", [T_CORE, TOP_K], F32, kind="ExternalOutput").ap()

    X = mybir.AxisListType.X
    Alu = mybir.AluOpType
    Act = mybir.ActivationFunctionType

    with TileContext(nc) as tc:
        with (
            tc.tile_pool(name="wpool", bufs=1) as wpool,
            tc.tile_pool(name="xpool", bufs=4) as xpool,
            tc.tile_pool(name="spool", bufs=2) as spool,
            tc.tile_pool(name="small", bufs=2) as small,
            tc.tile_pool(name="psmm", bufs=3, space="PSUM") as psmm,
            tc.tile_pool(name="pst0", bufs=1, space="PSUM") as pst0p,
            tc.tile_pool(name="pst1", bufs=1, space="PSUM") as pst1p,
        ):
            bias_sb = wpool.tile([128, E], F32, tag="bias")
            ident_sb = wpool.tile([128, 128], F32, tag="ident")
            WCH = 8
            NWC = KT // WCH  # 7
            wh_sb = [
                wpool.tile([128, WCH, E], F16, tag=f"wh{c}", name=f"wh{c}")
                for c in range(NWC)
            ]
            w8_sb = [
                wpool.tile([128, WCH, 2, E], F8, tag=f"w8_{c}", name=f"w8_{c}")
                for c in range(NWC)
            ]

            def wh_k(k, eh):
                return wh_sb[k // WCH][:, k % WCH, eh * 128 : (eh + 1) * 128]

            def w8_k(k, eh):
                return w8_sb[k // WCH][:, k % WCH, :, eh * 128 : (eh + 1) * 128]

            xh_sl = {}
            x8_sl = {}

            def slice_dma(c, s, quarters=1):
                tcn = CHUNKS[c]
                xh_t = xpool.tile([128, KS, 512], F16, tag="xh", name=f"xh{c}_{s}")
                x8_t = xpool.tile([128, 2, KS, 512], F8, tag="x8", name=f"x8{c}_{s}")
                xh_sl[(c, s)] = xh_t
                x8_sl[(c, s)] = x8_t
                ka = s * KS
                q = KS // quarters
                for j in range(quarters):
                    nc.sync.dma_start(
                        xh_t[:, j * q : (j + 1) * q, 0:tcn],
                        xh_d[c][:, ka + j * q : ka + (j + 1) * q, :],
                    )
                    nc.scalar.dma_start(
                        x8_t[:, 0, j * q : (j + 1) * q, 0:tcn],
                        xl8_d[c][:, ka + j * q : ka + (j + 1) * q, :],
                    )

            def slice_conv(c, s):
                tcn = CHUNKS[c]
                # on-chip xh8 = e4m3(xh16) into the resid-1 lane (ACT)
                nc.scalar.activation(
                    x8_sl[(c, s)][:, 1, :, 0:tcn],
                    xh_sl[(c, s)][:, :, 0:tcn],
                    Act.Copy,
                )

            # ---- ramp: weights + chunk 0 slices, need-ordered ----
            nc.sync.dma_start(wh_sb[0][:], wh_d[:, 0:WCH, :])
            nc.scalar.dma_start(w8_sb[0][:], w8_d[:, 0:WCH, :, :])
            slice_dma(0, 0, quarters=2)
            for c in range(1, NWC):
                nc.sync.dma_start(wh_sb[c][:], wh_d[:, c * WCH : (c + 1) * WCH, :])
                nc.scalar.dma_start(w8_sb[c][:], w8_d[:, c * WCH : (c + 1) * WCH, :, :])
                if c == 2:
                    slice_dma(0, 1, quarters=2)
                if c == 4:
                    slice_dma(0, 2)
            nc.scalar.dma_start(bias_sb[:], bias_d)
            nc.sync.dma_start(ident_sb[:], ident_d)
            slice_dma(0, 3)
            for s in range(N_SLICE):
                slice_conv(0, s)

            lg_eT = {}
            pst_tiles = {}

            def emit_mm_group(c, klo, khi, psm, psc):
                tcn = CHUNKS[c]
                for k in range(klo, khi):
                    s = k // KS
                    for eh in range(2):
                        nc.tensor.matmul(
                            psm[eh][:, 0:tcn], wh_k(k, eh),
                            xh_sl[(c, s)][:, k % KS, 0:tcn],
                            start=(k == 0), stop=(k == KT - 1),
                            skip_group_check=True,
                        )
                    for eh in range(2):
                        nc.tensor.matmul(
                            psc[eh][:, 0:tcn], w8_k(k, eh),
                            x8_sl[(c, s)][:, :, k % KS, 0:tcn],
                            start=(k == 0), stop=(k == KT - 1),
                            perf_mode=mybir.MatmulPerfMode.DoubleRow,
                            skip_group_check=True,
                        )

            def emit_transpose(c):
                bpc = CHUNKS[c] // 128
                for half in range((bpc + 1) // 2):
                    pool = pst0p if half == 0 else pst1p
                    pst = pool.tile([128, 2 * E], F32, tag="pst", name=f"pst{c}_{half}")
                    pst_tiles[(c, half)] = pst
                    for bi in range(min(2, bpc - 2 * half)):
                        b = half * 2 + bi
                        for eh in range(2):
                            nc.tensor.matmul(
                                pst[:, bi * E + eh * 128 : bi * E + (eh + 1) * 128],
                                lg_eT[(c, eh)][:, b * 128 : (b + 1) * 128],
                                ident_sb[:],
                                is_transpose=True,
                                start=(bi == 0 and eh == 0), stop=True,
                                skip_group_check=True,
                            )

            def emit_combine(c, psm, psc):
                tcn = CHUNKS[c]
                for eh in range(2):
                    psr = spool.tile([128, 512], F32, tag="psr")
                    nc.scalar.activation(
                        psr[:, 0:tcn], psc[eh][:, 0:tcn], Act.Copy, scale=1.0 / S12
                    )
                    lg = spool.tile([128, 512], F32, tag="lg", name=f"lg{c}_{eh}")
                    nc.vector.tensor_add(lg[:, 0:tcn], psm[eh][:, 0:tcn], psr[:, 0:tcn])
                    lg_eT[(c, eh)] = lg

            def emit_routing(c):
                for b in range(CHUNKS[c] // 128):
                    t0 = OFFS[c] + b * 128
                    lgT = pst_tiles[(c, b // 2)][:, (b % 2) * E : (b % 2 + 1) * E]
                    scores = spool.tile([128, E], F32, tag="scores")
                    nc.scalar.activation(
                        scores[:], lgT, Act.Sigmoid, scale=1.0 / W_SCALE
                    )
                    sfc = spool.tile([128, E], F32, tag="sfc")
                    nc.vector.tensor_add(sfc[:], scores[:], bias_sb[:])

                    sfc_g = sfc[:].rearrange("p (g e) -> p g e", g=N_GROUP)
                    g1 = small.tile([128, N_GROUP], F32, tag="g1")
                    nc.vector.reduce_max(g1[:], sfc_g, axis=X)
                    sfc_mr = spool.tile([128, E], F32, tag="scratch", name=f"smr{t0}")
                    nc.vector.match_replace(sfc_mr[:], g1[:], sfc[:], NEG_BIG)
                    g2 = small.tile([128, N_GROUP], F32, tag="g2")
                    nc.vector.reduce_max(
                        g2[:], sfc_mr[:].rearrange("p (g e) -> p g e", g=N_GROUP),
                        axis=X,
                    )
                    gs = small.tile([128, N_GROUP], F32, tag="gs")
                    nc.vector.tensor_add(gs[:], g1[:], g2[:])

                    gsrt = small.tile([128, 8], F32, tag="gsrt")
                    nc.vector.max(out=gsrt[:], in_=gs[:])
                    gmask = small.tile([128, N_GROUP], F32, tag="gmask")
                    nc.vector.tensor_scalar(
                        gmask[:], gs[:], gsrt[:, TOPK_GROUP - 1 : TOPK_GROUP], None,
                        op0=Alu.is_ge,
                    )

                    tmp = spool.tile([128, E], F32, tag="tmp")
                    nc.vector.tensor_mul(
                        tmp[:].rearrange("p (g e) -> p g e", g=N_GROUP),
                        sfc_g,
                        gmask[:].unsqueeze(2).to_broadcast([128, N_GROUP, GROUP_SIZE]),
                    )

                    v8 = small.tile([128, 8], F32, tag="v8")
                    nc.vector.max(out=v8[:], in_=tmp[:])
                    i8 = small.tile([128, 8], U32, tag="i8")
                    nc.vector.max_index(i8[:], v8[:], tmp[:])

                    tmp_mr = spool.tile([128, E], F32, tag="scratch", name=f"tmr{t0}")
                    nc.vector.match_replace(tmp_mr[:], v8[:], tmp[:], NEG_BIG)
                    sel = spool.tile([128, E], F32, tag="sel")
                    nc.vector.tensor_scalar(
                        sel[:], tmp_mr[:], NEG_BIG, None, op0=Alu.is_equal
                    )
                    scsel = spool.tile([128, E], F32, tag="scsel")
                    nc.vector.tensor_mul(scsel[:], scores[:], sel[:])
                    s8 = small.tile([128, 8], F32, tag="s8")
                    nc.vector.max(out=s8[:], in_=scsel[:])
                    s8i = small.tile([128, 8], U32, tag="s8i")
                    nc.vector.max_index(s8i[:], s8[:], scsel[:])

                    idx_out = small.tile([128, TOP_K], I32, tag="idx_out")
                    nc.vector.tensor_copy(idx_out[:], i8[:])
                    nc.sync.dma_start(idx_d[t0 : t0 + 128, :], idx_out[:])

                    e8 = small.tile([128, 8, 8], F32, tag="e8")
                    nc.vector.tensor_tensor(
                        e8[:],
                        s8i[:].unsqueeze(1).to_broadcast([128, 8, 8]),
                        i8[:].unsqueeze(2).to_broadcast([128, 8, 8]),
                        op=Alu.is_equal,
                    )
                    w64 = small.tile([128, 8, 8], F32, tag="w64")
                    nc.vector.tensor_mul(
                        w64[:], e8[:], s8[:].unsqueeze(1).to_broadcast([128, 8, 8])
                    )
                    w8v = small.tile([128, 8], F32, tag="w8v")
                    nc.vector.reduce_sum(w8v[:], w64[:], axis=X)

                    ds = small.tile([128, 1], F32, tag="ds")
                    nc.vector.reduce_sum(ds[:], s8[:], axis=X)
                    rcp = small.tile([128, 1], F32, tag="rcp")
                    nc.vector.reciprocal(rcp[:], ds[:])
                    w_out = small.tile([128, TOP_K], F32, tag="w_out")
                    nc.vector.tensor_scalar(
                        w_out[:], w8v[:], rcp[:, 0:1], ROUTED_SCALING,
                        op0=Alu.mult, op1=Alu.mult,
                    )
                    nc.sync.dma_start(w_d[t0 : t0 + 128, :], w_out[:])

            # ---- software pipeline over chunks ----
            for c in range(N_CHUNK):
                psm = [
                    psmm.tile([128, 512], F32, tag="psm", name=f"psm{c}_{eh}")
                    for eh in range(2)
                ]
                psc = [
                    psmm.tile([128, 512], F32, tag="psc", name=f"psc{c}_{eh}")
                    for eh in range(2)
                ]
                emit_mm_group(c, 0, 8, psm, psc)
                if c > 0:
                    emit_transpose(c - 1)
                emit_mm_group(c, 8, KT, psm, psc)
                if c > 0:
                    emit_routing(c - 1)
                if c + 1 < N_CHUNK:
                    for s in range(N_SLICE):
                        slice_dma(c + 1, s)
                    for s in range(N_SLICE):
                        slice_conv(c + 1, s)
                emit_combine(c, psm, psc)
            emit_transpose(N_CHUNK - 1)
            emit_routing(N_CHUNK - 1)

    nc.compile()
    return nc


_NC_CACHE = None


def _get_nc():
    global _NC_CACHE
    if _NC_CACHE is None:
        _NC_CACHE = _build_nc()
    return _NC_CACHE


def _prep_inputs(hidden_states, weight, e_score_correction_bias):
    x = np.ascontiguousarray(hidden_states, dtype=np.float32).reshape(T_FULL, H)
    wT = np.ascontiguousarray(weight, dtype=np.float32).T * W_SCALE  # [H, E]
    wh16 = wT.astype(np.float16)
    wl8 = ((wT - wh16.astype(np.float32)) * S12).astype(E4NP)
    wh8 = wh16.astype(E4NP)
    wh16_dev = np.ascontiguousarray(wh16.reshape(KT, 128, E).transpose(1, 0, 2))
    w8_dev = np.ascontiguousarray(
        np.stack(
            [wh8.reshape(KT, 128, E), wl8.reshape(KT, 128, E)], axis=2
        ).transpose(1, 0, 2, 3)
    )
    bias_b = np.ascontiguousarray(
        np.broadcast_to(
            np.asarray(e_score_correction_bias, dtype=np.float32)[None, :], (128, E)
        )
    )
    ident = np.eye(128, dtype=np.float32)
    in_maps = []
    for cr in range(N_CORES):
        xc = x[cr * T_CORE : (cr + 1) * T_CORE]  # [Tc, H]
        xh = xc.astype(np.float16)
        xl8 = ((xc - xh.astype(np.float32)) * S12).astype(E4NP)
        m = {
            "wh16": wh16_dev,
            "w8": w8_dev,
            "biasb": bias_b,
            "ident": ident,
        }
        for c in range(N_CHUNK):
            t0, tcn = OFFS[c], CHUNKS[c]
            # [p, k, t]: x[t0 + t, k*128 + p] -> A[p, k, t]
            m[f"xh{c}"] = np.ascontiguousarray(
                xh[t0 : t0 + tcn].reshape(tcn, KT, 128).transpose(2, 1, 0)
            )
            m[f"xl8_{c}"] = np.ascontiguousarray(
                xl8[t0 : t0 + tcn].reshape(tcn, KT, 128).transpose(2, 1, 0)
            )
        in_maps.append(m)
    return in_maps


def run(hidden_states, weight, e_score_correction_bias, trace=False, **spmd_kwargs):
    nc = _get_nc()
    in_maps = _prep_inputs(hidden_states, weight, e_score_correction_bias)
    res = run_bass_kernel_spmd(
        nc, in_maps, core_ids=list(range(N_CORES)), trace=trace, **spmd_kwargs
    )
    idx = np.concatenate([r["out_idx"] for r in res.results], axis=0)
    w = np.concatenate([r["out_w"] for r in res.results], axis=0)
    return (idx.astype(np.int32), w.astype(np.float32)), res


def kernel(hidden_states, weight, e_score_correction_bias):
    (idx, w), _ = run(hidden_states, weight, e_score_correction_bias, trace=False)
    return idx, w
